# revision 3
# baseline (speedup 1.0000x reference)
"""Trainium2 Bass kernel for nn_MoEBlock_30502857736769 (moe_routing).

Math (reference):
    out = sum_k v_k * relu(h @ wi^T + (h @ A_k^T) @ B_k^T) @ wo^T

Key algebraic restructuring (exact, since v_k >= 0 and wo is linear):
    base   = h @ wi^T                      (computed ONCE, shared by both experts)
    t_cat  = h @ [A0; A1; A0]^T            (rank-16 LoRA projections, one matmul)
    p      = base + l0                     (LoRA add via PSUM accumulation - free)
    act    = relu(v0*p) + relu(v1*(p + (l1 - l0)))   (l1-l0 added via one PSUM matmul)
    out    = act @ wo^T                    (applied ONCE to the weighted sum)

This halves the dominant matmul FLOPs vs. the reference (which runs the full
FFN per expert). Sharding: pure data-parallel over the 16384 tokens across the
8 NeuronCores (weights replicated); no collectives needed.

All layouts are pre-transposed on the host so every matmul operand is a
natural row-major slice. Matmuls run in fp16 (full PE rate; fp32 is 4x
slower), accumulating in fp32 PSUM.
"""

import numpy as np

# Problem constants (hardcoded per harness contract - no spec.json reads).
D_MODEL = 1024
D_FF = 4096
N_CORES = 8
B, S = 8, 2048
TOKENS = B * S            # 16384
T = TOKENS // N_CORES     # 2048 tokens per core

P = 128                   # SBUF/PE partition count


def _dt():
    import concourse.mybir as mybir
    return mybir.dt


def build_program(v0: float, v1: float, t_per_core: int = T, tc: int = 256):
    """Build + compile the SPMD single-core Bass program.

    DRAM parameter layouts (all fp16 except the fp32 output):
      xT  [D, Tc]   hidden-states shard, transposed (d-major)
      wiT [D, F]    wi^T
      woT [F, D]    wo^T
      aT  [D, 48]   [A_i0; A_i1; A_i0]^T   (three stacked rank-16 blocks)
      bTa [48, F]   [B_i0^T; 0; 0]         (adds l0 against t rows 0:16)
      bTb [48, F]   [0; B_i1^T; -B_i0^T]   (adds l1-l0 against t rows 16:48)
      out [Tc, D]   fp32 output shard (natural token-major layout)
    """
    import concourse.bass as bass
    import concourse.mybir as mybir
    import concourse.tile as tile
    from concourse import bacc
    from concourse.bass import ts, ds

    dt = mybir.dt
    AF = mybir.ActivationFunctionType

    D, F = D_MODEL, D_FF
    KD = D // P            # 8 contraction tiles over d_model
    KF = F // P            # 32 tiles over d_ff
    NCH = t_per_core // tc # token chunks
    TT = tc // P           # 128-token tiles per chunk
    MD = dt.float16

    assert t_per_core % tc == 0 and tc % P == 0

    nc = bacc.Bacc("TRN2", target_bir_lowering=False, debug=False)

    xT = nc.dram_tensor("xT", [D, t_per_core], MD, kind="ExternalInput")
    wiT = nc.dram_tensor("wiT", [D, F], MD, kind="ExternalInput")
    woT = nc.dram_tensor("woT", [F, D], MD, kind="ExternalInput")
    aT = nc.dram_tensor("aT", [D, 48], MD, kind="ExternalInput")
    bTa = nc.dram_tensor("bTa", [48, F], MD, kind="ExternalInput")
    bTb = nc.dram_tensor("bTb", [48, F], MD, kind="ExternalInput")
    out = nc.dram_tensor("out", [t_per_core, D], dt.float32, kind="ExternalOutput")

    with tile.TileContext(nc) as tc_ctx:
        with (
            tc_ctx.tile_pool(name="wi", bufs=1) as wi_pool,
            tc_ctx.tile_pool(name="wo", bufs=1) as wo_pool,
            tc_ctx.tile_pool(name="lora_w", bufs=1) as lw_pool,
            tc_ctx.tile_pool(name="x", bufs=2) as x_pool,
            tc_ctx.tile_pool(name="tcat", bufs=2) as tq_pool,
            tc_ctx.tile_pool(name="act", bufs=6) as act_pool,
            tc_ctx.tile_pool(name="a1", bufs=3) as a1_pool,
            tc_ctx.tile_pool(name="osb", bufs=3) as osb_pool,
            tc_ctx.tile_pool(name="ps1", bufs=3, space="PSUM") as ps1_pool,
            tc_ctx.tile_pool(name="pslora", bufs=1, space="PSUM") as pl_pool,
            tc_ctx.tile_pool(name="ps2", bufs=2, space="PSUM") as ps2_pool,
        ):
            # ---- resident weights ----
            wi_t = wi_pool.tile([P, KD, F], MD)    # wiT as KD tiles of [128, F]
            for kd in range(KD):
                nc.sync.dma_start(wi_t[:, kd, :], wiT[ts(kd, P), :])
            wo_t = wo_pool.tile([P, KF, D], MD)    # woT as KF tiles of [128, D]
            for kf in range(KF):
                nc.sync.dma_start(wo_t[:, kf, :], woT[ts(kf, P), :])
            a_t = lw_pool.tile([P, KD, 48], MD)
            for kd in range(KD):
                nc.sync.dma_start(a_t[:, kd, :], aT[ts(kd, P), :])
            bTa_t = lw_pool.tile([48, F], MD)
            nc.sync.dma_start(bTa_t[:, :], bTa[:, :])
            bTb_t = lw_pool.tile([48, F], MD)
            nc.sync.dma_start(bTb_t[:, :], bTb[:, :])

            for ch in range(NCH):
                # ---- load token chunk (transposed: [d, t]) ----
                x_t = x_pool.tile([P, KD, tc], MD, tag="x")
                for kd in range(KD):
                    nc.sync.dma_start(
                        x_t[:, kd, :], xT[ts(kd, P), ds(ch * tc, tc)]
                    )

                # ---- LoRA A projections: t_cat^T = [A0;A1;A0] @ x ----
                pl = pl_pool.tile([48, tc], dt.float32, tag="pslora")
                for kd in range(KD):
                    nc.tensor.matmul(
                        pl[:, :], a_t[:, kd, :], x_t[:, kd, :],
                        start=(kd == 0), stop=(kd == KD - 1),
                    )
                tq = tq_pool.tile([48, tc], MD, tag="tcat")
                nc.scalar.copy(tq[:, :], pl[:, :])

                # ---- stage-2 accumulators for this chunk ----
                ps2s = [
                    ps2_pool.tile([P, D], dt.float32, tag="ps2", name="ps2")
                    for _ in range(TT)
                ]

                for fi in range(KF):
                    # base^T tile = wi_fi @ x  (+ l0 via B_i0)
                    p1 = ps1_pool.tile([P, tc], dt.float32, tag="ps1")
                    for kd in range(KD):
                        nc.tensor.matmul(
                            p1[:, :], wi_t[:, kd, ts(fi, P)], x_t[:, kd, :],
                            start=(kd == 0), stop=False,
                        )
                    nc.tensor.matmul(
                        p1[:, :], bTa_t[:, ts(fi, P)], tq[:, :],
                        start=False, stop=True,
                    )
                    # act = relu(v0 * (base + l0))   [v0 >= 0]
                    act_t = act_pool.tile([P, tc], MD, tag="act")
                    nc.scalar.activation(
                        act_t[:, :], p1[:, :], AF.Relu, bias=0.0, scale=float(v0)
                    )
                    # p1 += l1 - l0: accumulate onto the closed group (the PSUM
                    # has_written bits persist; skip the sim's group bookkeeping)
                    nc.tensor.matmul(
                        p1[:, :], bTb_t[:, ts(fi, P)], tq[:, :],
                        start=False, stop=True, skip_group_check=True,
                    )
                    a1_t = a1_pool.tile([P, tc], MD, tag="a1")
                    nc.scalar.activation(
                        a1_t[:, :], p1[:, :], AF.Relu, bias=0.0, scale=float(v1)
                    )
                    nc.vector.tensor_add(act_t[:, :], act_t[:, :], a1_t[:, :])

                    # ---- fused stage 2: out[t,:] += act_fi^T-slice @ wo_fi ----
                    for tt in range(TT):
                        for dh in range(D // 512):  # one PSUM bank per matmul
                            nc.tensor.matmul(
                                ps2s[tt][:, ts(dh, 512)],
                                act_t[:, ts(tt, P)],
                                wo_t[:, fi, ts(dh, 512)],
                                start=(fi == 0), stop=(fi == KF - 1),
                            )

                # ---- evacuate + store this chunk ----
                for tt in range(TT):
                    osb = osb_pool.tile([P, D], dt.float32, tag="osb")
                    nc.vector.tensor_copy(osb[:, :], ps2s[tt][:, :])
                    nc.sync.dma_start(out[ds(ch * tc + tt * P, P), :], osb[:, :])

    nc.compile()
    return nc


_PROGRAM_CACHE = {}


def _get_program(v0: float, v1: float):
    key = (float(v0), float(v1))
    if key not in _PROGRAM_CACHE:
        _PROGRAM_CACHE[key] = build_program(v0, v1)
    return _PROGRAM_CACHE[key]


def prep_inputs(hidden_states, wi_w, wo_w, lora_As, lora_Bs,
                top_k_indices, top_k_values, t_per_core: int = T):
    """Host-side shard + layout prep. Returns (in_maps, v0, v1)."""
    h = np.ascontiguousarray(np.asarray(hidden_states, dtype=np.float32))
    wi = np.asarray(wi_w, dtype=np.float32)
    wo = np.asarray(wo_w, dtype=np.float32)
    As = np.asarray(lora_As, dtype=np.float32)
    Bs = np.asarray(lora_Bs, dtype=np.float32)
    idx = np.asarray(top_k_indices).astype(np.int64)
    vals = np.asarray(top_k_values, dtype=np.float32)

    i0, i1 = int(idx[0]), int(idx[1])
    v0, v1 = float(vals[0]), float(vals[1])

    wiT = np.ascontiguousarray(wi.T).astype(np.float16)          # [D, F]
    woT = np.ascontiguousarray(wo.T).astype(np.float16)          # [F, D]
    A0, A1 = As[i0], As[i1]                                      # [16, D]
    aT = np.ascontiguousarray(
        np.concatenate([A0, A1, A0], axis=0).T
    ).astype(np.float16)                                         # [D, 48]
    B0T, B1T = Bs[i0].T, Bs[i1].T                                # [16, F]
    bTa = np.zeros((48, D_FF), dtype=np.float16)
    bTa[0:16] = B0T.astype(np.float16)
    bTb = np.zeros((48, D_FF), dtype=np.float16)
    bTb[16:32] = B1T.astype(np.float16)
    bTb[32:48] = (-B0T).astype(np.float16)

    tokens = h.reshape(TOKENS, D_MODEL)
    n_cores = TOKENS // t_per_core
    in_maps = []
    for c in range(n_cores):
        shard = tokens[c * t_per_core:(c + 1) * t_per_core]
        xT = np.ascontiguousarray(shard.T).astype(np.float16)    # [D, Tc]
        in_maps.append({
            "xT": xT, "wiT": wiT, "woT": woT,
            "aT": aT, "bTa": bTa, "bTb": bTb,
        })
    return in_maps, v0, v1


# test.py can flip these to profile the run.
TRACE = False
TRACE_CORES = None
LAST_RESULT = None


def kernel(hidden_states, wi_w, wo_w, lora_As, lora_Bs,
           top_k_indices, top_k_values):
    global LAST_RESULT
    from concourse.bass_utils import run_bass_kernel_spmd

    in_maps, v0, v1 = prep_inputs(
        hidden_states, wi_w, wo_w, lora_As, lora_Bs,
        top_k_indices, top_k_values,
    )
    nc = _get_program(v0, v1)
    res = run_bass_kernel_spmd(
        nc, in_maps, list(range(N_CORES)),
        trace=TRACE, trace_cores=TRACE_CORES,
    )
    LAST_RESULT = res
    out = np.concatenate([r["out"] for r in res.results], axis=0)
    return out.reshape(B, S, D_MODEL).astype(np.float32, copy=False)


# revision 10
# speedup vs baseline: 1.1697x; 1.1697x over previous
"""Trainium2 Bass kernel for nn_MoEBlock_30502857736769 (moe_routing).

Math (reference):
    out = sum_k v_k * relu(h @ wi^T + (h @ A_k^T) @ B_k^T) @ wo^T

Key algebraic restructuring (exact, since v_k >= 0 and wo is linear):
    base   = h @ wi^T                      (computed ONCE, shared by both experts)
    t_cat  = h @ [A0; A1; A0]^T            (rank-16 LoRA projections, one matmul)
    p      = base + l0                     (LoRA add via PSUM accumulation - free)
    act    = relu(v0*p) + relu(v1*(p + (l1 - l0)))   (l1-l0 added via one PSUM matmul)
    out    = act @ wo^T                    (applied ONCE to the weighted sum)

This halves the dominant matmul FLOPs vs. the reference (which runs the full
FFN per expert). Sharding: pure data-parallel over the 16384 tokens across the
8 NeuronCores (weights replicated); no collectives needed.

All layouts are pre-transposed on the host so every matmul operand is a
natural row-major slice. Matmuls run in fp16 (full PE rate; fp32 is 4x
slower), accumulating in fp32 PSUM.
"""

import numpy as np

# Problem constants (hardcoded per harness contract - no spec.json reads).
D_MODEL = 1024
D_FF = 4096
N_CORES = 8
B, S = 8, 2048
TOKENS = B * S            # 16384
T = TOKENS // N_CORES     # 2048 tokens per core

P = 128                   # SBUF/PE partition count


def _dt():
    import concourse.mybir as mybir
    return mybir.dt


def build_program(v0: float, v1: float, t_per_core: int = T, tc: int = 256):
    """Build + compile the SPMD single-core Bass program.

    DRAM parameter layouts (all fp16 except the fp32 output):
      xT  [D, Tc]   hidden-states shard, transposed (d-major)
      wiT [D, F]    wi^T
      woT [F, D]    wo^T
      aT  [D, 48]   [A_i0; A_i1; A_i0]^T   (three stacked rank-16 blocks)
      bTa [128, F]  [B_i0^T; 0...]         (adds l0 against t rows 0:16)
      bTb [128, F]  [0; B_i1^T; -B_i0^T; 0...]  (adds l1-l0, t rows 16:48)
    The B weights are zero-padded to K=128 so every stage-1 matmul has a
    full-row-extent LDWEIGHTS (K=48 loads conflict with in-flight full-row
    matmuls and serialize at ~2x spacing - measured on HW).
      out [Tc, D]   fp32 output shard (natural token-major layout)
    """
    import concourse.bass as bass
    import concourse.mybir as mybir
    import concourse.tile as tile
    from concourse import bacc
    from concourse.bass import ts, ds

    dt = mybir.dt
    AF = mybir.ActivationFunctionType

    D, F = D_MODEL, D_FF
    KD = D // P            # 8 contraction tiles over d_model
    KF = F // P            # 32 tiles over d_ff
    NCH = t_per_core // tc # token chunks
    TT = tc // P           # 128-token tiles per chunk
    MD = dt.float16

    assert t_per_core % tc == 0 and tc % P == 0

    nc = bacc.Bacc("TRN2", target_bir_lowering=False, debug=False)

    xT = nc.dram_tensor("xT", [D, t_per_core], MD, kind="ExternalInput")
    wiT = nc.dram_tensor("wiT", [D, F], MD, kind="ExternalInput")
    woT = nc.dram_tensor("woT", [F, D], MD, kind="ExternalInput")
    aT = nc.dram_tensor("aT", [D, 48], MD, kind="ExternalInput")
    bTa = nc.dram_tensor("bTa", [P, F], MD, kind="ExternalInput")
    bTb = nc.dram_tensor("bTb", [P, F], MD, kind="ExternalInput")
    out = nc.dram_tensor("out", [t_per_core, D], dt.float32, kind="ExternalOutput")
    AOT = mybir.AluOpType

    with tile.TileContext(nc) as tc_ctx:
        with (
            tc_ctx.tile_pool(name="wi", bufs=1) as wi_pool,
            tc_ctx.tile_pool(name="wo", bufs=1) as wo_pool,
            tc_ctx.tile_pool(name="lora_w", bufs=1) as lw_pool,
            tc_ctx.tile_pool(name="x", bufs=2) as x_pool,
            tc_ctx.tile_pool(name="tcat", bufs=2) as tq_pool,
            tc_ctx.tile_pool(name="act", bufs=6) as act_pool,
            tc_ctx.tile_pool(name="a1", bufs=3) as a1_pool,
            tc_ctx.tile_pool(name="osb", bufs=3) as osb_pool,
            tc_ctx.tile_pool(name="ps1", bufs=3, space="PSUM") as ps1_pool,
            tc_ctx.tile_pool(name="pslora", bufs=1, space="PSUM") as pl_pool,
            tc_ctx.tile_pool(name="ps2", bufs=2, space="PSUM") as ps2_pool,
        ):
            # ---- DMA order: everything chunk 0 needs first, then the bulk
            #      weights (16 MB), so compute starts ~45us sooner.
            a_t = lw_pool.tile([P, KD, 48], MD)
            for kd in range(KD):
                nc.sync.dma_start(a_t[:, kd, :], aT[ts(kd, P), :])
            bTa_t = lw_pool.tile([P, F], MD)
            nc.sync.dma_start(bTa_t[:, :], bTa[:, :])
            bTb_t = lw_pool.tile([P, F], MD)
            nc.sync.dma_start(bTb_t[:, :], bTb[:, :])
            x0_t = x_pool.tile([P, KD, tc], MD, tag="x", name="x_t")
            for kd in range(KD):
                nc.sync.dma_start(x0_t[:, kd, :], xT[ts(kd, P), ds(0, tc)])
            wi_t = wi_pool.tile([P, KD, F], MD)    # wiT as KD tiles of [128, F]
            for kd in range(KD):
                nc.sync.dma_start(wi_t[:, kd, :], wiT[ts(kd, P), :])
            wo_t = wo_pool.tile([P, KF, D], MD)    # woT as KF tiles of [128, D]
            for kf in range(KF):
                nc.sync.dma_start(wo_t[:, kf, :], woT[ts(kf, P), :])

            for ch in range(NCH):
                # ---- load token chunk (transposed: [d, t]) ----
                if ch == 0:
                    x_t = x0_t
                else:
                    x_t = x_pool.tile([P, KD, tc], MD, tag="x", name="x_t")
                    for kd in range(KD):
                        nc.sync.dma_start(
                            x_t[:, kd, :], xT[ts(kd, P), ds(ch * tc, tc)]
                        )

                # ---- LoRA A projections: t_cat^T = [A0;A1;A0] @ x ----
                pl = pl_pool.tile([48, tc], dt.float32, tag="pslora")
                for kd in range(KD):
                    nc.tensor.matmul(
                        pl[:, :], a_t[:, kd, :], x_t[:, kd, :],
                        start=(kd == 0), stop=(kd == KD - 1),
                    )
                tq = tq_pool.tile([P, tc], MD, tag="tcat")
                nc.gpsimd.memset(tq[:, :], 0.0)
                nc.scalar.copy(tq[0:48, :], pl[:, :])

                # ---- stage-2 accumulators for this chunk ----
                ps2s = [
                    ps2_pool.tile([P, D], dt.float32, tag="ps2", name="ps2")
                    for _ in range(TT)
                ]

                for fi in range(KF):
                    # base^T tile = wi_fi @ x  (+ l0 via B_i0)
                    p1 = ps1_pool.tile([P, tc], dt.float32, tag="ps1")
                    for kd in range(KD):
                        nc.tensor.matmul(
                            p1[:, :], wi_t[:, kd, ts(fi, P)], x_t[:, kd, :],
                            start=(kd == 0), stop=False,
                        )
                    nc.tensor.matmul(
                        p1[:, :], bTa_t[:, ts(fi, P)], tq[:, :],
                        start=False, stop=True,
                    )
                    # act = relu(v0 * (base + l0))   [v0 >= 0]
                    act_t = act_pool.tile([P, tc], MD, tag="act")
                    nc.scalar.activation(
                        act_t[:, :], p1[:, :], AF.Relu, bias=0.0, scale=float(v0)
                    )
                    # p1 += l1 - l0: accumulate onto the closed group (the PSUM
                    # has_written bits persist; skip the sim's group bookkeeping)
                    nc.tensor.matmul(
                        p1[:, :], bTb_t[:, ts(fi, P)], tq[:, :],
                        start=False, stop=True, skip_group_check=True,
                    )
                    # a1 = max(p1, 0) * v1 on DVE (keeps ACT off the
                    # relu0->Bdiff critical chain)
                    a1_t = a1_pool.tile([P, tc], MD, tag="a1")
                    nc.vector.tensor_scalar(
                        a1_t[:, :], p1[:, :], 0.0, float(v1),
                        AOT.max, AOT.mult,
                    )
                    nc.vector.tensor_add(act_t[:, :], act_t[:, :], a1_t[:, :])

                    # ---- fused stage 2: out[t,:] += act_fi^T-slice @ wo_fi ----
                    for tt in range(TT):
                        for dh in range(D // 512):  # one PSUM bank per matmul
                            nc.tensor.matmul(
                                ps2s[tt][:, ts(dh, 512)],
                                act_t[:, ts(tt, P)],
                                wo_t[:, fi, ts(dh, 512)],
                                start=(fi == 0), stop=(fi == KF - 1),
                            )

                # ---- evacuate + store this chunk ----
                for tt in range(TT):
                    osb = osb_pool.tile([P, D], dt.float32, tag="osb")
                    nc.vector.tensor_copy(osb[:, :], ps2s[tt][:, :])
                    nc.sync.dma_start(out[ds(ch * tc + tt * P, P), :], osb[:, :])

    nc.compile()
    return nc


_PROGRAM_CACHE = {}


def _get_program(v0: float, v1: float):
    key = (float(v0), float(v1))
    if key not in _PROGRAM_CACHE:
        _PROGRAM_CACHE[key] = build_program(v0, v1)
    return _PROGRAM_CACHE[key]


def prep_inputs(hidden_states, wi_w, wo_w, lora_As, lora_Bs,
                top_k_indices, top_k_values, t_per_core: int = T):
    """Host-side shard + layout prep. Returns (in_maps, v0, v1)."""
    h = np.ascontiguousarray(np.asarray(hidden_states, dtype=np.float32))
    wi = np.asarray(wi_w, dtype=np.float32)
    wo = np.asarray(wo_w, dtype=np.float32)
    As = np.asarray(lora_As, dtype=np.float32)
    Bs = np.asarray(lora_Bs, dtype=np.float32)
    idx = np.asarray(top_k_indices).astype(np.int64)
    vals = np.asarray(top_k_values, dtype=np.float32)

    i0, i1 = int(idx[0]), int(idx[1])
    v0, v1 = float(vals[0]), float(vals[1])

    wiT = np.ascontiguousarray(wi.T).astype(np.float16)          # [D, F]
    woT = np.ascontiguousarray(wo.T).astype(np.float16)          # [F, D]
    A0, A1 = As[i0], As[i1]                                      # [16, D]
    aT = np.ascontiguousarray(
        np.concatenate([A0, A1, A0], axis=0).T
    ).astype(np.float16)                                         # [D, 48]
    B0T, B1T = Bs[i0].T, Bs[i1].T                                # [16, F]
    bTa = np.zeros((128, D_FF), dtype=np.float16)
    bTa[0:16] = B0T.astype(np.float16)
    bTb = np.zeros((128, D_FF), dtype=np.float16)
    bTb[16:32] = B1T.astype(np.float16)
    bTb[32:48] = (-B0T).astype(np.float16)

    tokens = h.reshape(TOKENS, D_MODEL)
    n_cores = TOKENS // t_per_core
    in_maps = []
    for c in range(n_cores):
        shard = tokens[c * t_per_core:(c + 1) * t_per_core]
        xT = np.ascontiguousarray(shard.T).astype(np.float16)    # [D, Tc]
        in_maps.append({
            "xT": xT, "wiT": wiT, "woT": woT,
            "aT": aT, "bTa": bTa, "bTb": bTb,
        })
    return in_maps, v0, v1


# test.py can flip these to profile the run.
TRACE = False
TRACE_CORES = None
LAST_RESULT = None


def kernel(hidden_states, wi_w, wo_w, lora_As, lora_Bs,
           top_k_indices, top_k_values):
    global LAST_RESULT
    from concourse.bass_utils import run_bass_kernel_spmd

    in_maps, v0, v1 = prep_inputs(
        hidden_states, wi_w, wo_w, lora_As, lora_Bs,
        top_k_indices, top_k_values,
    )
    nc = _get_program(v0, v1)
    res = run_bass_kernel_spmd(
        nc, in_maps, list(range(N_CORES)),
        trace=TRACE, trace_cores=TRACE_CORES,
    )
    LAST_RESULT = res
    out = np.concatenate([r["out"] for r in res.results], axis=0)
    return out.reshape(B, S, D_MODEL).astype(np.float32, copy=False)


# revision 13
# speedup vs baseline: 1.1941x; 1.0208x over previous
"""Trainium2 Bass kernel for nn_MoEBlock_30502857736769 (moe_routing).

Math (reference):
    out = sum_k v_k * relu(h @ wi^T + (h @ A_k^T) @ B_k^T) @ wo^T

Key algebraic restructuring (exact, since v_k >= 0 and wo is linear):
    base   = h @ wi^T                      (computed ONCE, shared by both experts)
    t_cat  = h @ [A0; A1; A0]^T            (rank-16 LoRA projections, one matmul)
    p      = base + l0                     (LoRA add via PSUM accumulation - free)
    act    = relu(v0*p) + relu(v1*(p + (l1 - l0)))   (l1-l0 added via one PSUM matmul)
    out    = act @ wo^T                    (applied ONCE to the weighted sum)

This halves the dominant matmul FLOPs vs. the reference (which runs the full
FFN per expert). Sharding: pure data-parallel over the 16384 tokens across the
8 NeuronCores (weights replicated); no collectives needed.

All layouts are pre-transposed on the host so every matmul operand is a
natural row-major slice. Matmuls run in fp16 (full PE rate; fp32 is 4x
slower), accumulating in fp32 PSUM.
"""

import numpy as np

# Problem constants (hardcoded per harness contract - no spec.json reads).
D_MODEL = 1024
D_FF = 4096
N_CORES = 8
B, S = 8, 2048
TOKENS = B * S            # 16384
T = TOKENS // N_CORES     # 2048 tokens per core

P = 128                   # SBUF/PE partition count


def _dt():
    import concourse.mybir as mybir
    return mybir.dt


def build_program(v0: float, v1: float, t_per_core: int = T, tc: int = 256):
    """Build + compile the SPMD single-core Bass program.

    DRAM parameter layouts (all fp16 except the fp32 output):
      xT  [D, Tc]   hidden-states shard, transposed (d-major)
      wiT [D, F]    wi^T
      woT [F, D]    wo^T
      aT  [D, 48]   [A_i0; A_i1; A_i0]^T   (three stacked rank-16 blocks)
      bTa [128, F]  [B_i0^T; 0...]         (adds l0 against t rows 0:16)
      bTb [128, F]  [0; B_i1^T; -B_i0^T; 0...]  (adds l1-l0, t rows 16:48)
    The B weights are zero-padded to K=128 so every stage-1 matmul has a
    full-row-extent LDWEIGHTS (K=48 loads conflict with in-flight full-row
    matmuls and serialize at ~2x spacing - measured on HW).
      out [Tc, D]   fp32 output shard (natural token-major layout)
    """
    import concourse.bass as bass
    import concourse.mybir as mybir
    import concourse.tile as tile
    from concourse import bacc
    from concourse.bass import ts, ds

    dt = mybir.dt
    AF = mybir.ActivationFunctionType

    D, F = D_MODEL, D_FF
    KD = D // P            # 8 contraction tiles over d_model
    KF = F // P            # 32 tiles over d_ff
    NCH = t_per_core // tc # token chunks
    TT = tc // P           # 128-token tiles per chunk
    MD = dt.float16

    assert t_per_core % tc == 0 and tc % P == 0

    nc = bacc.Bacc("TRN2", target_bir_lowering=False, debug=False)

    xT = nc.dram_tensor("xT", [D, t_per_core], MD, kind="ExternalInput")
    wiT = nc.dram_tensor("wiT", [D, F], MD, kind="ExternalInput")
    woT = nc.dram_tensor("woT", [F, D], MD, kind="ExternalInput")
    aT = nc.dram_tensor("aT", [D, 48], MD, kind="ExternalInput")
    bTa = nc.dram_tensor("bTa", [P, F], MD, kind="ExternalInput")
    bTb = nc.dram_tensor("bTb", [P, F], MD, kind="ExternalInput")
    out = nc.dram_tensor("out", [t_per_core, D], dt.float32, kind="ExternalOutput")
    AOT = mybir.AluOpType

    with tile.TileContext(nc) as tc_ctx:
        with (
            tc_ctx.tile_pool(name="wi", bufs=1) as wi_pool,
            tc_ctx.tile_pool(name="wo", bufs=1) as wo_pool,
            tc_ctx.tile_pool(name="lora_w", bufs=1) as lw_pool,
            tc_ctx.tile_pool(name="x", bufs=2) as x_pool,
            tc_ctx.tile_pool(name="tcat", bufs=2) as tq_pool,
            tc_ctx.tile_pool(name="act", bufs=6) as act_pool,
            tc_ctx.tile_pool(name="a1", bufs=3) as a1_pool,
            tc_ctx.tile_pool(name="osb", bufs=3) as osb_pool,
            tc_ctx.tile_pool(name="ps1", bufs=3, space="PSUM") as ps1_pool,
            tc_ctx.tile_pool(name="pslora", bufs=1, space="PSUM") as pl_pool,
            tc_ctx.tile_pool(name="ps2", bufs=2, space="PSUM") as ps2_pool,
        ):
            # ---- DMA order: everything chunk 0 needs first, then the bulk
            #      weights (16 MB), so compute starts ~45us sooner.
            a_t = lw_pool.tile([P, KD, 48], MD)
            for kd in range(KD):
                nc.sync.dma_start(a_t[:, kd, :], aT[ts(kd, P), :])
            x0_t = x_pool.tile([P, KD, tc], MD, tag="x", name="x_t")
            for kd in range(KD):
                nc.sync.dma_start(x0_t[:, kd, :], xT[ts(kd, P), ds(0, tc)])
            bTa_t = lw_pool.tile([P, F], MD)
            nc.sync.dma_start(bTa_t[:, :], bTa[:, :])
            bTb_t = lw_pool.tile([P, F], MD)
            nc.sync.dma_start(bTb_t[:, :], bTb[:, :])
            # wi streamed in F-quarters (k-major inside a quarter) so chunk 0's
            # early f-tiles can start before the full 8 MB lands
            wi_t = wi_pool.tile([P, KD, F], MD)    # wiT as KD tiles of [128, F]
            FQ = F // 4
            for q in range(4):
                for kd in range(KD):
                    nc.sync.dma_start(
                        wi_t[:, kd, ds(q * FQ, FQ)],
                        wiT[ts(kd, P), ds(q * FQ, FQ)],
                    )
            wo_t = wo_pool.tile([P, KF, D], MD)    # woT as KF tiles of [128, D]
            for kf in range(KF):
                nc.sync.dma_start(wo_t[:, kf, :], woT[ts(kf, P), :])

            for ch in range(NCH):
                # ---- load token chunk (transposed: [d, t]) ----
                if ch == 0:
                    x_t = x0_t
                else:
                    x_t = x_pool.tile([P, KD, tc], MD, tag="x", name="x_t")
                    for kd in range(KD):
                        nc.sync.dma_start(
                            x_t[:, kd, :], xT[ts(kd, P), ds(ch * tc, tc)]
                        )

                # ---- LoRA A projections: t_cat^T = [A0;A1;A0] @ x ----
                pl = pl_pool.tile([48, tc], dt.float32, tag="pslora")
                for kd in range(KD):
                    nc.tensor.matmul(
                        pl[:, :], a_t[:, kd, :], x_t[:, kd, :],
                        start=(kd == 0), stop=(kd == KD - 1),
                    )
                tq = tq_pool.tile([P, tc], MD, tag="tcat")
                nc.gpsimd.memset(tq[:, :], 0.0)
                nc.scalar.copy(tq[0:48, :], pl[:, :])

                # ---- stage-2 accumulators for this chunk ----
                ps2s = [
                    ps2_pool.tile([P, D], dt.float32, tag="ps2", name="ps2")
                    for _ in range(TT)
                ]

                # stage-2 matmuls for f-tile i are emitted during iteration
                # i+1 so the DVE add feeding their LDWEIGHTS has slack
                pending_s2 = None

                def emit_s2(act_prev, fi_prev):
                    for tt in range(TT):
                        for dh in range(D // 512):
                            nc.tensor.matmul(
                                ps2s[tt][:, ts(dh, 512)],
                                act_prev[:, ts(tt, P)],
                                wo_t[:, fi_prev, ts(dh, 512)],
                                start=(fi_prev == 0), stop=(fi_prev == KF - 1),
                            )

                for fi in range(KF):
                    # base^T tile = wi_fi @ x  (+ l0 via B_i0)
                    p1 = ps1_pool.tile([P, tc], dt.float32, tag="ps1")
                    for kd in range(KD):
                        nc.tensor.matmul(
                            p1[:, :], wi_t[:, kd, ts(fi, P)], x_t[:, kd, :],
                            start=(kd == 0), stop=False,
                        )
                    nc.tensor.matmul(
                        p1[:, :], bTa_t[:, ts(fi, P)], tq[:, :],
                        start=False, stop=True,
                    )
                    # act = relu(v0 * (base + l0))   [v0 >= 0]
                    act_t = act_pool.tile([P, tc], MD, tag="act")
                    nc.scalar.activation(
                        act_t[:, :], p1[:, :], AF.Relu, bias=0.0, scale=float(v0)
                    )
                    # p1 += l1 - l0: accumulate onto the closed group (the PSUM
                    # has_written bits persist; skip the sim's group bookkeeping)
                    nc.tensor.matmul(
                        p1[:, :], bTb_t[:, ts(fi, P)], tq[:, :],
                        start=False, stop=True, skip_group_check=True,
                    )
                    # a1 = max(p1, 0) * v1 on DVE (keeps ACT off the
                    # relu0->Bdiff critical chain)
                    a1_t = a1_pool.tile([P, tc], MD, tag="a1")
                    nc.vector.tensor_scalar(
                        a1_t[:, :], p1[:, :], 0.0, float(v1),
                        AOT.max, AOT.mult,
                    )
                    nc.vector.tensor_add(act_t[:, :], act_t[:, :], a1_t[:, :])

                    # ---- fused stage 2 (delayed one f-tile) ----
                    if pending_s2 is not None:
                        emit_s2(*pending_s2)
                    pending_s2 = (act_t, fi)
                emit_s2(*pending_s2)

                # ---- evacuate + store this chunk ----
                for tt in range(TT):
                    osb = osb_pool.tile([P, D], dt.float32, tag="osb")
                    nc.vector.tensor_copy(osb[:, :], ps2s[tt][:, :])
                    nc.sync.dma_start(out[ds(ch * tc + tt * P, P), :], osb[:, :])

    nc.compile()
    return nc


_PROGRAM_CACHE = {}


def _get_program(v0: float, v1: float):
    key = (float(v0), float(v1))
    if key not in _PROGRAM_CACHE:
        _PROGRAM_CACHE[key] = build_program(v0, v1)
    return _PROGRAM_CACHE[key]


def prep_inputs(hidden_states, wi_w, wo_w, lora_As, lora_Bs,
                top_k_indices, top_k_values, t_per_core: int = T):
    """Host-side shard + layout prep. Returns (in_maps, v0, v1)."""
    h = np.ascontiguousarray(np.asarray(hidden_states, dtype=np.float32))
    wi = np.asarray(wi_w, dtype=np.float32)
    wo = np.asarray(wo_w, dtype=np.float32)
    As = np.asarray(lora_As, dtype=np.float32)
    Bs = np.asarray(lora_Bs, dtype=np.float32)
    idx = np.asarray(top_k_indices).astype(np.int64)
    vals = np.asarray(top_k_values, dtype=np.float32)

    i0, i1 = int(idx[0]), int(idx[1])
    v0, v1 = float(vals[0]), float(vals[1])

    wiT = np.ascontiguousarray(wi.T).astype(np.float16)          # [D, F]
    woT = np.ascontiguousarray(wo.T).astype(np.float16)          # [F, D]
    A0, A1 = As[i0], As[i1]                                      # [16, D]
    aT = np.ascontiguousarray(
        np.concatenate([A0, A1, A0], axis=0).T
    ).astype(np.float16)                                         # [D, 48]
    B0T, B1T = Bs[i0].T, Bs[i1].T                                # [16, F]
    bTa = np.zeros((128, D_FF), dtype=np.float16)
    bTa[0:16] = B0T.astype(np.float16)
    bTb = np.zeros((128, D_FF), dtype=np.float16)
    bTb[16:32] = B1T.astype(np.float16)
    bTb[32:48] = (-B0T).astype(np.float16)

    tokens = h.reshape(TOKENS, D_MODEL)
    n_cores = TOKENS // t_per_core
    in_maps = []
    for c in range(n_cores):
        shard = tokens[c * t_per_core:(c + 1) * t_per_core]
        xT = np.ascontiguousarray(shard.T).astype(np.float16)    # [D, Tc]
        in_maps.append({
            "xT": xT, "wiT": wiT, "woT": woT,
            "aT": aT, "bTa": bTa, "bTb": bTb,
        })
    return in_maps, v0, v1


# test.py can flip these to profile the run.
TRACE = False
TRACE_CORES = None
LAST_RESULT = None


def kernel(hidden_states, wi_w, wo_w, lora_As, lora_Bs,
           top_k_indices, top_k_values):
    global LAST_RESULT
    from concourse.bass_utils import run_bass_kernel_spmd

    in_maps, v0, v1 = prep_inputs(
        hidden_states, wi_w, wo_w, lora_As, lora_Bs,
        top_k_indices, top_k_values,
    )
    nc = _get_program(v0, v1)
    res = run_bass_kernel_spmd(
        nc, in_maps, list(range(N_CORES)),
        trace=TRACE, trace_cores=TRACE_CORES,
    )
    LAST_RESULT = res
    out = np.concatenate([r["out"] for r in res.results], axis=0)
    return out.reshape(B, S, D_MODEL).astype(np.float32, copy=False)


# revision 16
# speedup vs baseline: 1.2031x; 1.0076x over previous
"""Trainium2 Bass kernel for nn_MoEBlock_30502857736769 (moe_routing).

Math (reference):
    out = sum_k v_k * relu(h @ wi^T + (h @ A_k^T) @ B_k^T) @ wo^T

Key algebraic restructuring (exact, since v_k >= 0 and wo is linear):
    base   = h @ wi^T                      (computed ONCE, shared by both experts)
    t_cat  = h @ [A0; A1; A0]^T            (rank-16 LoRA projections, one matmul)
    p      = base + l0                     (LoRA add via PSUM accumulation - free)
    act    = relu(v0*p) + relu(v1*(p + (l1 - l0)))   (l1-l0 added via one PSUM matmul)
    out    = act @ wo^T                    (applied ONCE to the weighted sum)

This halves the dominant matmul FLOPs vs. the reference (which runs the full
FFN per expert). Sharding: pure data-parallel over the 16384 tokens across the
8 NeuronCores (weights replicated); no collectives needed.

All layouts are pre-transposed on the host so every matmul operand is a
natural row-major slice. Matmuls run in fp16 (full PE rate; fp32 is 4x
slower), accumulating in fp32 PSUM.
"""

import numpy as np

# Problem constants (hardcoded per harness contract - no spec.json reads).
D_MODEL = 1024
D_FF = 4096
N_CORES = 8
B, S = 8, 2048
TOKENS = B * S            # 16384
T = TOKENS // N_CORES     # 2048 tokens per core

P = 128                   # SBUF/PE partition count


def _dt():
    import concourse.mybir as mybir
    return mybir.dt


def build_program(v0: float, v1: float, t_per_core: int = T, tc: int = 256):
    """Build + compile the SPMD single-core Bass program.

    DRAM parameter layouts (all fp16 except the fp32 output):
      xT  [D, Tc]   hidden-states shard, transposed (d-major)
      wiT [D, F]    wi^T
      woT [F, D]    wo^T
      aT  [D, 48]   [A_i0; A_i1; A_i0]^T   (three stacked rank-16 blocks)
      bTa [128, F]  [B_i0^T; 0...]         (adds l0 against t rows 0:16)
      bTb [128, F]  [0; B_i1^T; -B_i0^T; 0...]  (adds l1-l0, t rows 16:48)
    The B weights are zero-padded to K=128 so every stage-1 matmul has a
    full-row-extent LDWEIGHTS (K=48 loads conflict with in-flight full-row
    matmuls and serialize at ~2x spacing - measured on HW).
      out [Tc, D]   fp32 output shard (natural token-major layout)
    """
    import concourse.bass as bass
    import concourse.mybir as mybir
    import concourse.tile as tile
    from concourse import bacc
    from concourse.bass import ts, ds

    dt = mybir.dt
    AF = mybir.ActivationFunctionType

    D, F = D_MODEL, D_FF
    KD = D // P            # 8 contraction tiles over d_model
    KF = F // P            # 32 tiles over d_ff
    NCH = t_per_core // tc # token chunks
    TT = tc // P           # 128-token tiles per chunk
    MD = dt.float16

    assert t_per_core % tc == 0 and tc % P == 0

    nc = bacc.Bacc("TRN2", target_bir_lowering=False, debug=False)

    xT = nc.dram_tensor("xT", [D, t_per_core], MD, kind="ExternalInput")
    wiT = nc.dram_tensor("wiT", [D, F], MD, kind="ExternalInput")
    woT = nc.dram_tensor("woT", [F, D], MD, kind="ExternalInput")
    aT = nc.dram_tensor("aT", [D, 48], MD, kind="ExternalInput")
    bTa = nc.dram_tensor("bTa", [P, F], MD, kind="ExternalInput")
    bTb = nc.dram_tensor("bTb", [P, F], MD, kind="ExternalInput")
    out = nc.dram_tensor("out", [t_per_core, D], dt.float32, kind="ExternalOutput")
    AOT = mybir.AluOpType

    with tile.TileContext(nc) as tc_ctx:
        with (
            tc_ctx.tile_pool(name="wi", bufs=1) as wi_pool,
            tc_ctx.tile_pool(name="wo", bufs=1) as wo_pool,
            tc_ctx.tile_pool(name="lora_w", bufs=1) as lw_pool,
            tc_ctx.tile_pool(name="x", bufs=2) as x_pool,
            tc_ctx.tile_pool(name="tcat", bufs=2) as tq_pool,
            tc_ctx.tile_pool(name="act", bufs=6) as act_pool,
            tc_ctx.tile_pool(name="a1", bufs=3) as a1_pool,
            tc_ctx.tile_pool(name="osb", bufs=3) as osb_pool,
            tc_ctx.tile_pool(name="ps1", bufs=3, space="PSUM") as ps1_pool,
            tc_ctx.tile_pool(name="pslora", bufs=1, space="PSUM") as pl_pool,
            tc_ctx.tile_pool(name="ps2", bufs=2, space="PSUM") as ps2_pool,
        ):
            # ---- DMA order: everything chunk 0 needs first, then the bulk
            #      weights (16 MB), so compute starts ~45us sooner.
            a_t = lw_pool.tile([P, KD, 48], MD)
            for kd in range(KD):
                nc.sync.dma_start(a_t[:, kd, :], aT[ts(kd, P), :])
            x0_t = x_pool.tile([P, KD, tc], MD, tag="x", name="x_t")
            for kd in range(KD):
                nc.sync.dma_start(x0_t[:, kd, :], xT[ts(kd, P), ds(0, tc)])
            bTa_t = lw_pool.tile([P, F], MD)
            nc.sync.dma_start(bTa_t[:, :], bTa[:, :])
            bTb_t = lw_pool.tile([P, F], MD)
            nc.sync.dma_start(bTb_t[:, :], bTb[:, :])
            # wi streamed in F-quarters (k-major inside a quarter) interleaved
            # with the wo tiles those f-tiles' stage-2 needs, so chunk 0's
            # compute tracks DMA arrival instead of stalling on the tail
            wi_t = wi_pool.tile([P, KD, F], MD)    # wiT as KD tiles of [128, F]
            wo_t = wo_pool.tile([P, KF, D], MD)    # woT as KF tiles of [128, D]
            FQ = F // 4
            for q in range(4):
                for kd in range(KD):
                    nc.sync.dma_start(
                        wi_t[:, kd, ds(q * FQ, FQ)],
                        wiT[ts(kd, P), ds(q * FQ, FQ)],
                    )
                for kf in range(q * KF // 4, (q + 1) * KF // 4):
                    nc.sync.dma_start(wo_t[:, kf, :], woT[ts(kf, P), :])

            for ch in range(NCH):
                # ---- load token chunk (transposed: [d, t]) ----
                if ch == 0:
                    x_t = x0_t
                else:
                    x_t = x_pool.tile([P, KD, tc], MD, tag="x", name="x_t")
                    for kd in range(KD):
                        nc.sync.dma_start(
                            x_t[:, kd, :], xT[ts(kd, P), ds(ch * tc, tc)]
                        )

                # ---- LoRA A projections: t_cat^T = [A0;A1;A0] @ x ----
                pl = pl_pool.tile([48, tc], dt.float32, tag="pslora")
                for kd in range(KD):
                    nc.tensor.matmul(
                        pl[:, :], a_t[:, kd, :], x_t[:, kd, :],
                        start=(kd == 0), stop=(kd == KD - 1),
                    )
                tq = tq_pool.tile([P, tc], MD, tag="tcat")
                nc.gpsimd.memset(tq[:, :], 0.0)
                nc.scalar.copy(tq[0:48, :], pl[:, :])

                # ---- stage-2 accumulators for this chunk ----
                ps2s = [
                    ps2_pool.tile([P, D], dt.float32, tag="ps2", name="ps2")
                    for _ in range(TT)
                ]

                # stage-2 matmuls for f-tile i are emitted during iteration
                # i+1 so the DVE add feeding their LDWEIGHTS has slack
                pending_s2 = None

                def emit_s2(act_prev, fi_prev):
                    for tt in range(TT):
                        for dh in range(D // 512):
                            nc.tensor.matmul(
                                ps2s[tt][:, ts(dh, 512)],
                                act_prev[:, ts(tt, P)],
                                wo_t[:, fi_prev, ts(dh, 512)],
                                start=(fi_prev == 0), stop=(fi_prev == KF - 1),
                            )

                for fi in range(KF):
                    # base^T tile = wi_fi @ x  (+ l0 via B_i0)
                    p1 = ps1_pool.tile([P, tc], dt.float32, tag="ps1")
                    for kd in range(KD):
                        nc.tensor.matmul(
                            p1[:, :], wi_t[:, kd, ts(fi, P)], x_t[:, kd, :],
                            start=(kd == 0), stop=False,
                        )
                    nc.tensor.matmul(
                        p1[:, :], bTa_t[:, ts(fi, P)], tq[:, :],
                        start=False, stop=True,
                    )
                    # act = relu(v0 * (base + l0))   [v0 >= 0]
                    act_t = act_pool.tile([P, tc], MD, tag="act")
                    nc.scalar.activation(
                        act_t[:, :], p1[:, :], AF.Relu, bias=0.0, scale=float(v0)
                    )
                    # stage 2 for the previous f-tile goes here: its 4 N=512
                    # matmuls cover the B0->relu0 latency before Bdiff issues
                    if pending_s2 is not None:
                        emit_s2(*pending_s2)
                        pending_s2 = None
                    # p1 += l1 - l0: accumulate onto the closed group (the PSUM
                    # has_written bits persist; skip the sim's group bookkeeping)
                    nc.tensor.matmul(
                        p1[:, :], bTb_t[:, ts(fi, P)], tq[:, :],
                        start=False, stop=True, skip_group_check=True,
                    )
                    # a1 = max(p1, 0) * v1 on DVE (keeps ACT off the
                    # relu0->Bdiff critical chain)
                    a1_t = a1_pool.tile([P, tc], MD, tag="a1")
                    nc.vector.tensor_scalar(
                        a1_t[:, :], p1[:, :], 0.0, float(v1),
                        AOT.max, AOT.mult,
                    )
                    nc.vector.tensor_add(act_t[:, :], act_t[:, :], a1_t[:, :])
                    pending_s2 = (act_t, fi)
                emit_s2(*pending_s2)

                # ---- evacuate + store this chunk ----
                for tt in range(TT):
                    osb = osb_pool.tile([P, D], dt.float32, tag="osb")
                    nc.vector.tensor_copy(osb[:, :], ps2s[tt][:, :])
                    nc.sync.dma_start(out[ds(ch * tc + tt * P, P), :], osb[:, :])

    nc.compile()
    return nc


_PROGRAM_CACHE = {}


def _get_program(v0: float, v1: float):
    key = (float(v0), float(v1))
    if key not in _PROGRAM_CACHE:
        _PROGRAM_CACHE[key] = build_program(v0, v1)
    return _PROGRAM_CACHE[key]


def prep_inputs(hidden_states, wi_w, wo_w, lora_As, lora_Bs,
                top_k_indices, top_k_values, t_per_core: int = T):
    """Host-side shard + layout prep. Returns (in_maps, v0, v1)."""
    h = np.ascontiguousarray(np.asarray(hidden_states, dtype=np.float32))
    wi = np.asarray(wi_w, dtype=np.float32)
    wo = np.asarray(wo_w, dtype=np.float32)
    As = np.asarray(lora_As, dtype=np.float32)
    Bs = np.asarray(lora_Bs, dtype=np.float32)
    idx = np.asarray(top_k_indices).astype(np.int64)
    vals = np.asarray(top_k_values, dtype=np.float32)

    i0, i1 = int(idx[0]), int(idx[1])
    v0, v1 = float(vals[0]), float(vals[1])

    wiT = np.ascontiguousarray(wi.T).astype(np.float16)          # [D, F]
    woT = np.ascontiguousarray(wo.T).astype(np.float16)          # [F, D]
    A0, A1 = As[i0], As[i1]                                      # [16, D]
    aT = np.ascontiguousarray(
        np.concatenate([A0, A1, A0], axis=0).T
    ).astype(np.float16)                                         # [D, 48]
    B0T, B1T = Bs[i0].T, Bs[i1].T                                # [16, F]
    bTa = np.zeros((128, D_FF), dtype=np.float16)
    bTa[0:16] = B0T.astype(np.float16)
    bTb = np.zeros((128, D_FF), dtype=np.float16)
    bTb[16:32] = B1T.astype(np.float16)
    bTb[32:48] = (-B0T).astype(np.float16)

    tokens = h.reshape(TOKENS, D_MODEL)
    n_cores = TOKENS // t_per_core
    in_maps = []
    for c in range(n_cores):
        shard = tokens[c * t_per_core:(c + 1) * t_per_core]
        xT = np.ascontiguousarray(shard.T).astype(np.float16)    # [D, Tc]
        in_maps.append({
            "xT": xT, "wiT": wiT, "woT": woT,
            "aT": aT, "bTa": bTa, "bTb": bTb,
        })
    return in_maps, v0, v1


# test.py can flip these to profile the run.
TRACE = False
TRACE_CORES = None
LAST_RESULT = None


def kernel(hidden_states, wi_w, wo_w, lora_As, lora_Bs,
           top_k_indices, top_k_values):
    global LAST_RESULT
    from concourse.bass_utils import run_bass_kernel_spmd

    in_maps, v0, v1 = prep_inputs(
        hidden_states, wi_w, wo_w, lora_As, lora_Bs,
        top_k_indices, top_k_values,
    )
    nc = _get_program(v0, v1)
    res = run_bass_kernel_spmd(
        nc, in_maps, list(range(N_CORES)),
        trace=TRACE, trace_cores=TRACE_CORES,
    )
    LAST_RESULT = res
    out = np.concatenate([r["out"] for r in res.results], axis=0)
    return out.reshape(B, S, D_MODEL).astype(np.float32, copy=False)


# revision 17
# speedup vs baseline: 1.2754x; 1.0601x over previous
"""Trainium2 Bass kernel for nn_MoEBlock_30502857736769 (moe_routing).

Math (reference):
    out = sum_k v_k * relu(h @ wi^T + (h @ A_k^T) @ B_k^T) @ wo^T

Key algebraic restructuring (exact, since v_k >= 0 and wo is linear):
    base   = h @ wi^T                      (computed ONCE, shared by both experts)
    t_cat  = h @ [A0; A1; A0]^T            (rank-16 LoRA projections, one matmul)
    p      = base + l0                     (LoRA add via PSUM accumulation - free)
    act    = relu(v0*p) + relu(v1*(p + (l1 - l0)))   (l1-l0 added via one PSUM matmul)
    out    = act @ wo^T                    (applied ONCE to the weighted sum)

This halves the dominant matmul FLOPs vs. the reference (which runs the full
FFN per expert). Sharding: pure data-parallel over the 16384 tokens across the
8 NeuronCores (weights replicated); no collectives needed.

All layouts are pre-transposed on the host so every matmul operand is a
natural row-major slice. Matmuls run in fp16 (full PE rate; fp32 is 4x
slower), accumulating in fp32 PSUM.
"""

import numpy as np

# Problem constants (hardcoded per harness contract - no spec.json reads).
D_MODEL = 1024
D_FF = 4096
N_CORES = 8
B, S = 8, 2048
TOKENS = B * S            # 16384
T = TOKENS // N_CORES     # 2048 tokens per core

P = 128                   # SBUF/PE partition count


def _dt():
    import concourse.mybir as mybir
    return mybir.dt


def build_program(v0: float, v1: float, t_per_core: int = T, tc: int = 256):
    """Build + compile the SPMD single-core Bass program.

    DRAM parameter layouts (all fp16 except the fp32 output):
      xT  [D, Tc]   hidden-states shard, transposed (d-major)
      wiT [D, F]    wi^T
      woT [F, D]    wo^T
      aT  [D, 48]   [A_i0; A_i1; A_i0]^T   (three stacked rank-16 blocks)
      bTa [128, F]  [B_i0^T; 0...]         (adds l0 against t rows 0:16)
      bTb [128, F]  [0; B_i1^T; -B_i0^T; 0...]  (adds l1-l0, t rows 16:48)
    The B weights are zero-padded to K=128 so every stage-1 matmul has a
    full-row-extent LDWEIGHTS (K=48 loads conflict with in-flight full-row
    matmuls and serialize at ~2x spacing - measured on HW).
      out [Tc, D]   fp32 output shard (natural token-major layout)
    """
    import concourse.bass as bass
    import concourse.mybir as mybir
    import concourse.tile as tile
    from concourse import bacc
    from concourse.bass import ts, ds

    dt = mybir.dt
    AF = mybir.ActivationFunctionType

    D, F = D_MODEL, D_FF
    KD = D // P            # 8 contraction tiles over d_model
    KF = F // P            # 32 tiles over d_ff
    NCH = t_per_core // tc # token chunks
    TT = tc // P           # 128-token tiles per chunk
    MD = dt.float16

    assert t_per_core % tc == 0 and tc % P == 0

    nc = bacc.Bacc("TRN2", target_bir_lowering=False, debug=False)

    xT = nc.dram_tensor("xT", [D, t_per_core], MD, kind="ExternalInput")
    wiT = nc.dram_tensor("wiT", [D, F], MD, kind="ExternalInput")
    woT = nc.dram_tensor("woT", [F, D], MD, kind="ExternalInput")
    aT = nc.dram_tensor("aT", [D, 48], MD, kind="ExternalInput")
    bTa = nc.dram_tensor("bTa", [P, F], MD, kind="ExternalInput")
    bTb = nc.dram_tensor("bTb", [P, F], MD, kind="ExternalInput")
    out = nc.dram_tensor("out", [t_per_core, D], dt.float32, kind="ExternalOutput")
    AOT = mybir.AluOpType

    with tile.TileContext(nc) as tc_ctx:
        with (
            tc_ctx.tile_pool(name="wi", bufs=1) as wi_pool,
            tc_ctx.tile_pool(name="wo", bufs=1) as wo_pool,
            tc_ctx.tile_pool(name="lora_w", bufs=1) as lw_pool,
            tc_ctx.tile_pool(name="x", bufs=2) as x_pool,
            tc_ctx.tile_pool(name="tcat", bufs=2) as tq_pool,
            tc_ctx.tile_pool(name="act", bufs=6) as act_pool,
            tc_ctx.tile_pool(name="a1", bufs=3) as a1_pool,
            tc_ctx.tile_pool(name="osb", bufs=3) as osb_pool,
            tc_ctx.tile_pool(name="ps1", bufs=3, space="PSUM") as ps1_pool,
            tc_ctx.tile_pool(name="pslora", bufs=1, space="PSUM") as pl_pool,
            tc_ctx.tile_pool(name="ps2", bufs=2, space="PSUM") as ps2_pool,
        ):
            # ---- DMA order: everything chunk 0 needs first, then the bulk
            #      weights (16 MB), so compute starts ~45us sooner.
            a_t = lw_pool.tile([P, KD, 48], MD)
            for kd in range(KD):
                nc.sync.dma_start(a_t[:, kd, :], aT[ts(kd, P), :])
            x0_t = x_pool.tile([P, KD, tc], MD, tag="x", name="x_t")
            for kd in range(KD):
                nc.sync.dma_start(x0_t[:, kd, :], xT[ts(kd, P), ds(0, tc)])
            bTa_t = lw_pool.tile([P, F], MD)
            nc.sync.dma_start(bTa_t[:, :], bTa[:, :])
            bTb_t = lw_pool.tile([P, F], MD)
            nc.sync.dma_start(bTb_t[:, :], bTb[:, :])
            # wi streamed in F-quarters (k-major inside a quarter) interleaved
            # with the wo tiles those f-tiles' stage-2 needs, so chunk 0's
            # compute tracks DMA arrival instead of stalling on the tail
            wi_t = wi_pool.tile([P, KD, F], MD)    # wiT as KD tiles of [128, F]
            wo_t = wo_pool.tile([P, KF, D], MD)    # woT as KF tiles of [128, D]
            FQ = F // 4
            for q in range(4):
                for kd in range(KD):
                    nc.sync.dma_start(
                        wi_t[:, kd, ds(q * FQ, FQ)],
                        wiT[ts(kd, P), ds(q * FQ, FQ)],
                    )
                for kf in range(q * KF // 4, (q + 1) * KF // 4):
                    nc.sync.dma_start(wo_t[:, kf, :], woT[ts(kf, P), :])

            for ch in range(NCH):
                # ---- load token chunk (transposed: [d, t]) ----
                if ch == 0:
                    x_t = x0_t
                else:
                    x_t = x_pool.tile([P, KD, tc], MD, tag="x", name="x_t")
                    for kd in range(KD):
                        nc.sync.dma_start(
                            x_t[:, kd, :], xT[ts(kd, P), ds(ch * tc, tc)]
                        )

                # ---- LoRA A projections: t_cat^T = [A0;A1;A0] @ x ----
                pl = pl_pool.tile([48, tc], dt.float32, tag="pslora")
                for kd in range(KD):
                    nc.tensor.matmul(
                        pl[:, :], a_t[:, kd, :], x_t[:, kd, :],
                        start=(kd == 0), stop=(kd == KD - 1),
                    )
                tq = tq_pool.tile([P, tc], MD, tag="tcat")
                nc.gpsimd.memset(tq[:, :], 0.0)
                nc.scalar.copy(tq[0:48, :], pl[:, :])

                # ---- stage-2 accumulators for this chunk ----
                ps2s = [
                    ps2_pool.tile([P, D], dt.float32, tag="ps2", name="ps2")
                    for _ in range(TT)
                ]

                # Two-deep software pipeline over f-tiles:
                #   iter i emits:  s1 matmuls (wi x8 + B0) for f-tile i,
                #                  relu0(i) on ACT,
                #                  stage-2 matmuls for f-tile i-2,
                #                  Bdiff + relu1-path (DVE) for f-tile i-1.
                # This gives the B0(i)->relu0(i)->Bdiff(i) chain ~1.8us of
                # independent PE work as cover, so the PE never waits on ACT.
                def emit_s2(act_prev, fi_prev):
                    for tt in range(TT):
                        for dh in range(D // 512):
                            nc.tensor.matmul(
                                ps2s[tt][:, ts(dh, 512)],
                                act_prev[:, ts(tt, P)],
                                wo_t[:, fi_prev, ts(dh, 512)],
                                start=(fi_prev == 0), stop=(fi_prev == KF - 1),
                            )

                def emit_bdiff(st):
                    p1_, act_, fi_ = st
                    nc.tensor.matmul(
                        p1_[:, :], bTb_t[:, ts(fi_, P)], tq[:, :],
                        start=False, stop=True, skip_group_check=True,
                    )
                    a1_t = a1_pool.tile([P, tc], MD, tag="a1", name="a1_t")
                    nc.vector.tensor_scalar(
                        a1_t[:, :], p1_[:, :], 0.0, float(v1),
                        AOT.max, AOT.mult,
                    )
                    nc.vector.tensor_add(act_[:, :], act_[:, :], a1_t[:, :])

                prev = None       # (p1, act, fi) of f-tile i-1
                s2q = []          # acts awaiting stage-2 emission
                for fi in range(KF):
                    # base^T tile = wi_fi @ x  (+ l0 via B_i0)
                    p1 = ps1_pool.tile([P, tc], dt.float32, tag="ps1")
                    for kd in range(KD):
                        nc.tensor.matmul(
                            p1[:, :], wi_t[:, kd, ts(fi, P)], x_t[:, kd, :],
                            start=(kd == 0), stop=False,
                        )
                    nc.tensor.matmul(
                        p1[:, :], bTa_t[:, ts(fi, P)], tq[:, :],
                        start=False, stop=True,
                    )
                    # act = relu(v0 * (base + l0))   [v0 >= 0]
                    act_t = act_pool.tile([P, tc], MD, tag="act")
                    nc.scalar.activation(
                        act_t[:, :], p1[:, :], AF.Relu, bias=0.0, scale=float(v0)
                    )
                    if len(s2q) >= 2:
                        emit_s2(*s2q.pop(0))
                    if prev is not None:
                        emit_bdiff(prev)
                        s2q.append((prev[1], prev[2]))
                    prev = (p1, act_t, fi)
                # drain the pipeline
                emit_bdiff(prev)
                s2q.append((prev[1], prev[2]))
                for item in s2q:
                    emit_s2(*item)

                # ---- evacuate + store this chunk ----
                for tt in range(TT):
                    osb = osb_pool.tile([P, D], dt.float32, tag="osb")
                    nc.vector.tensor_copy(osb[:, :], ps2s[tt][:, :])
                    nc.sync.dma_start(out[ds(ch * tc + tt * P, P), :], osb[:, :])

    nc.compile()
    return nc


_PROGRAM_CACHE = {}


def _get_program(v0: float, v1: float):
    key = (float(v0), float(v1))
    if key not in _PROGRAM_CACHE:
        _PROGRAM_CACHE[key] = build_program(v0, v1)
    return _PROGRAM_CACHE[key]


def prep_inputs(hidden_states, wi_w, wo_w, lora_As, lora_Bs,
                top_k_indices, top_k_values, t_per_core: int = T):
    """Host-side shard + layout prep. Returns (in_maps, v0, v1)."""
    h = np.ascontiguousarray(np.asarray(hidden_states, dtype=np.float32))
    wi = np.asarray(wi_w, dtype=np.float32)
    wo = np.asarray(wo_w, dtype=np.float32)
    As = np.asarray(lora_As, dtype=np.float32)
    Bs = np.asarray(lora_Bs, dtype=np.float32)
    idx = np.asarray(top_k_indices).astype(np.int64)
    vals = np.asarray(top_k_values, dtype=np.float32)

    i0, i1 = int(idx[0]), int(idx[1])
    v0, v1 = float(vals[0]), float(vals[1])

    wiT = np.ascontiguousarray(wi.T).astype(np.float16)          # [D, F]
    woT = np.ascontiguousarray(wo.T).astype(np.float16)          # [F, D]
    A0, A1 = As[i0], As[i1]                                      # [16, D]
    aT = np.ascontiguousarray(
        np.concatenate([A0, A1, A0], axis=0).T
    ).astype(np.float16)                                         # [D, 48]
    B0T, B1T = Bs[i0].T, Bs[i1].T                                # [16, F]
    bTa = np.zeros((128, D_FF), dtype=np.float16)
    bTa[0:16] = B0T.astype(np.float16)
    bTb = np.zeros((128, D_FF), dtype=np.float16)
    bTb[16:32] = B1T.astype(np.float16)
    bTb[32:48] = (-B0T).astype(np.float16)

    tokens = h.reshape(TOKENS, D_MODEL)
    n_cores = TOKENS // t_per_core
    in_maps = []
    for c in range(n_cores):
        shard = tokens[c * t_per_core:(c + 1) * t_per_core]
        xT = np.ascontiguousarray(shard.T).astype(np.float16)    # [D, Tc]
        in_maps.append({
            "xT": xT, "wiT": wiT, "woT": woT,
            "aT": aT, "bTa": bTa, "bTb": bTb,
        })
    return in_maps, v0, v1


# test.py can flip these to profile the run.
TRACE = False
TRACE_CORES = None
LAST_RESULT = None


def kernel(hidden_states, wi_w, wo_w, lora_As, lora_Bs,
           top_k_indices, top_k_values):
    global LAST_RESULT
    from concourse.bass_utils import run_bass_kernel_spmd

    in_maps, v0, v1 = prep_inputs(
        hidden_states, wi_w, wo_w, lora_As, lora_Bs,
        top_k_indices, top_k_values,
    )
    nc = _get_program(v0, v1)
    res = run_bass_kernel_spmd(
        nc, in_maps, list(range(N_CORES)),
        trace=TRACE, trace_cores=TRACE_CORES,
    )
    LAST_RESULT = res
    out = np.concatenate([r["out"] for r in res.results], axis=0)
    return out.reshape(B, S, D_MODEL).astype(np.float32, copy=False)


# revision 19
# speedup vs baseline: 1.2978x; 1.0175x over previous
"""Trainium2 Bass kernel for nn_MoEBlock_30502857736769 (moe_routing).

Math (reference):
    out = sum_k v_k * relu(h @ wi^T + (h @ A_k^T) @ B_k^T) @ wo^T

Key algebraic restructuring (exact, since v_k >= 0 and wo is linear):
    base   = h @ wi^T                      (computed ONCE, shared by both experts)
    t_cat  = h @ [A0; A1; A0]^T            (rank-16 LoRA projections, one matmul)
    p      = base + l0                     (LoRA add via PSUM accumulation - free)
    act    = relu(v0*p) + relu(v1*(p + (l1 - l0)))   (l1-l0 added via one PSUM matmul)
    out    = act @ wo^T                    (applied ONCE to the weighted sum)

This halves the dominant matmul FLOPs vs. the reference (which runs the full
FFN per expert). Sharding: pure data-parallel over the 16384 tokens across the
8 NeuronCores (weights replicated); no collectives needed.

All layouts are pre-transposed on the host so every matmul operand is a
natural row-major slice. Matmuls run in fp16 (full PE rate; fp32 is 4x
slower), accumulating in fp32 PSUM.
"""

import numpy as np

# Problem constants (hardcoded per harness contract - no spec.json reads).
D_MODEL = 1024
D_FF = 4096
N_CORES = 8
B, S = 8, 2048
TOKENS = B * S            # 16384
T = TOKENS // N_CORES     # 2048 tokens per core

P = 128                   # SBUF/PE partition count


def _dt():
    import concourse.mybir as mybir
    return mybir.dt


def build_program(v0: float, v1: float, t_per_core: int = T, tc: int = 256):
    """Build + compile the SPMD single-core Bass program.

    DRAM parameter layouts (all fp16 except the fp32 output):
      xT  [D, Tc]   hidden-states shard, transposed (d-major)
      wiT [D, F]    wi^T
      woT [F, D]    wo^T
      aT  [D, 48]   [A_i0; A_i1; A_i0]^T   (three stacked rank-16 blocks)
      bTa [128, F]  [B_i0^T; 0...]         (adds l0 against t rows 0:16)
      bTb [128, F]  [0; B_i1^T; -B_i0^T; 0...]  (adds l1-l0, t rows 16:48)
    The B weights are zero-padded to K=128 so every stage-1 matmul has a
    full-row-extent LDWEIGHTS (K=48 loads conflict with in-flight full-row
    matmuls and serialize at ~2x spacing - measured on HW).
      out [Tc, D]   fp32 output shard (natural token-major layout)
    """
    import concourse.bass as bass
    import concourse.mybir as mybir
    import concourse.tile as tile
    from concourse import bacc
    from concourse.bass import ts, ds

    dt = mybir.dt
    AF = mybir.ActivationFunctionType

    D, F = D_MODEL, D_FF
    KD = D // P            # 8 contraction tiles over d_model
    KF = F // P            # 32 tiles over d_ff
    NCH = t_per_core // tc # token chunks
    TT = tc // P           # 128-token tiles per chunk
    MD = dt.float16

    assert t_per_core % tc == 0 and tc % P == 0

    nc = bacc.Bacc("TRN2", target_bir_lowering=False, debug=False)

    xT = nc.dram_tensor("xT", [D, t_per_core], MD, kind="ExternalInput")
    wiT = nc.dram_tensor("wiT", [D, F], MD, kind="ExternalInput")
    woT = nc.dram_tensor("woT", [F, D], MD, kind="ExternalInput")
    aT = nc.dram_tensor("aT", [D, 48], MD, kind="ExternalInput")
    bTa = nc.dram_tensor("bTa", [P, F], MD, kind="ExternalInput")
    bTb = nc.dram_tensor("bTb", [P, F], MD, kind="ExternalInput")
    out = nc.dram_tensor("out", [t_per_core, D], dt.float32, kind="ExternalOutput")
    AOT = mybir.AluOpType

    with tile.TileContext(nc) as tc_ctx:
        with (
            tc_ctx.tile_pool(name="wi", bufs=1) as wi_pool,
            tc_ctx.tile_pool(name="wo", bufs=1) as wo_pool,
            tc_ctx.tile_pool(name="lora_w", bufs=1) as lw_pool,
            tc_ctx.tile_pool(name="x", bufs=2) as x_pool,
            tc_ctx.tile_pool(name="tcat", bufs=2) as tq_pool,
            tc_ctx.tile_pool(name="act", bufs=6) as act_pool,
            tc_ctx.tile_pool(name="a1", bufs=3) as a1_pool,
            tc_ctx.tile_pool(name="osb", bufs=3) as osb_pool,
            tc_ctx.tile_pool(name="ps1", bufs=3, space="PSUM") as ps1_pool,
            tc_ctx.tile_pool(name="pslora", bufs=1, space="PSUM") as pl_pool,
            tc_ctx.tile_pool(name="ps2", bufs=2, space="PSUM") as ps2_pool,
        ):
            # ---- DMA order: everything chunk 0 needs first, then the bulk
            #      weights (16 MB), so compute starts ~45us sooner.
            # Single-trigger DMAs (rearranged APs) in earliest-deadline order:
            # chunk 0 consumes ~0.39 MB/us while HBM supplies ~0.36, so the
            # stream order must track demand (wi eighth j feeds f-tiles 4j..,
            # wo[f] feeds the f-tile's stage 2 two iterations later).
            a_t = lw_pool.tile([P, KD, 48], MD)
            nc.sync.dma_start(
                a_t[:, :, :], aT[:, :].rearrange("(kd p) r -> p kd r", p=P)
            )
            x0_t = x_pool.tile([P, KD, tc], MD, tag="x", name="x_t")
            nc.sync.dma_start(
                x0_t[:, :, :],
                xT[:, ds(0, tc)].rearrange("(kd p) t -> p kd t", p=P),
            )
            bTa_t = lw_pool.tile([P, F], MD)
            nc.sync.dma_start(bTa_t[:, :], bTa[:, :])
            wi_t = wi_pool.tile([P, KD, F], MD)    # wiT as KD tiles of [128, F]
            wo_t = wo_pool.tile([P, KF, D], MD)    # woT as KF tiles of [128, D]
            FE = F // 8

            def wi_eighth(j):
                nc.sync.dma_start(
                    wi_t[:, :, ds(j * FE, FE)],
                    wiT[:, ds(j * FE, FE)].rearrange("(kd p) f -> p kd f", p=P),
                )

            def wo_tile(kf):
                nc.sync.dma_start(wo_t[:, kf, :], woT[ts(kf, P), :])

            wi_eighth(0)
            bTb_t = lw_pool.tile([P, F], MD)
            nc.sync.dma_start(bTb_t[:, :], bTb[:, :])
            wi_eighth(1)
            next_wo = 0
            for j in range(2, 8):
                for _ in range(3):
                    wo_tile(next_wo); next_wo += 1
                wi_eighth(j)
            while next_wo < KF:
                wo_tile(next_wo); next_wo += 1

            for ch in range(NCH):
                # ---- load token chunk (transposed: [d, t]) ----
                if ch == 0:
                    x_t = x0_t
                else:
                    x_t = x_pool.tile([P, KD, tc], MD, tag="x", name="x_t")
                    nc.sync.dma_start(
                        x_t[:, :, :],
                        xT[:, ds(ch * tc, tc)].rearrange(
                            "(kd p) t -> p kd t", p=P
                        ),
                    )

                # ---- LoRA A projections: t_cat^T = [A0;A1;A0] @ x ----
                pl = pl_pool.tile([48, tc], dt.float32, tag="pslora")
                for kd in range(KD):
                    nc.tensor.matmul(
                        pl[:, :], a_t[:, kd, :], x_t[:, kd, :],
                        start=(kd == 0), stop=(kd == KD - 1),
                    )
                tq = tq_pool.tile([P, tc], MD, tag="tcat")
                nc.gpsimd.memset(tq[:, :], 0.0)
                nc.scalar.copy(tq[0:48, :], pl[:, :])

                # ---- stage-2 accumulators for this chunk ----
                ps2s = [
                    ps2_pool.tile([P, D], dt.float32, tag="ps2", name="ps2")
                    for _ in range(TT)
                ]

                # Two-deep software pipeline over f-tiles:
                #   iter i emits:  s1 matmuls (wi x8 + B0) for f-tile i,
                #                  relu0(i) on ACT,
                #                  stage-2 matmuls for f-tile i-2,
                #                  Bdiff + relu1-path (DVE) for f-tile i-1.
                # This gives the B0(i)->relu0(i)->Bdiff(i) chain ~1.8us of
                # independent PE work as cover, so the PE never waits on ACT.
                def emit_s2(act_prev, fi_prev):
                    for tt in range(TT):
                        for dh in range(D // 512):
                            nc.tensor.matmul(
                                ps2s[tt][:, ts(dh, 512)],
                                act_prev[:, ts(tt, P)],
                                wo_t[:, fi_prev, ts(dh, 512)],
                                start=(fi_prev == 0), stop=(fi_prev == KF - 1),
                            )

                def emit_bdiff(st):
                    p1_, act_, fi_ = st
                    nc.tensor.matmul(
                        p1_[:, :], bTb_t[:, ts(fi_, P)], tq[:, :],
                        start=False, stop=True, skip_group_check=True,
                    )
                    a1_t = a1_pool.tile([P, tc], MD, tag="a1", name="a1_t")
                    nc.vector.tensor_scalar(
                        a1_t[:, :], p1_[:, :], 0.0, float(v1),
                        AOT.max, AOT.mult,
                    )
                    nc.vector.tensor_add(act_[:, :], act_[:, :], a1_t[:, :])

                prev = None       # (p1, act, fi) of f-tile i-1
                s2q = []          # acts awaiting stage-2 emission
                for fi in range(KF):
                    # base^T tile = wi_fi @ x  (+ l0 via B_i0)
                    p1 = ps1_pool.tile([P, tc], dt.float32, tag="ps1")
                    for kd in range(KD):
                        nc.tensor.matmul(
                            p1[:, :], wi_t[:, kd, ts(fi, P)], x_t[:, kd, :],
                            start=(kd == 0), stop=False,
                        )
                    nc.tensor.matmul(
                        p1[:, :], bTa_t[:, ts(fi, P)], tq[:, :],
                        start=False, stop=True,
                    )
                    # act = relu(v0 * (base + l0))   [v0 >= 0]
                    act_t = act_pool.tile([P, tc], MD, tag="act")
                    nc.scalar.activation(
                        act_t[:, :], p1[:, :], AF.Relu, bias=0.0, scale=float(v0)
                    )
                    if len(s2q) >= 2:
                        emit_s2(*s2q.pop(0))
                    if prev is not None:
                        emit_bdiff(prev)
                        s2q.append((prev[1], prev[2]))
                    prev = (p1, act_t, fi)
                # drain the pipeline
                emit_bdiff(prev)
                s2q.append((prev[1], prev[2]))
                for item in s2q:
                    emit_s2(*item)

                # ---- evacuate + store this chunk ----
                for tt in range(TT):
                    osb = osb_pool.tile([P, D], dt.float32, tag="osb")
                    nc.vector.tensor_copy(osb[:, :], ps2s[tt][:, :])
                    nc.sync.dma_start(out[ds(ch * tc + tt * P, P), :], osb[:, :])

    nc.compile()
    return nc


_PROGRAM_CACHE = {}


def _get_program(v0: float, v1: float):
    key = (float(v0), float(v1))
    if key not in _PROGRAM_CACHE:
        _PROGRAM_CACHE[key] = build_program(v0, v1)
    return _PROGRAM_CACHE[key]


def prep_inputs(hidden_states, wi_w, wo_w, lora_As, lora_Bs,
                top_k_indices, top_k_values, t_per_core: int = T):
    """Host-side shard + layout prep. Returns (in_maps, v0, v1)."""
    h = np.ascontiguousarray(np.asarray(hidden_states, dtype=np.float32))
    wi = np.asarray(wi_w, dtype=np.float32)
    wo = np.asarray(wo_w, dtype=np.float32)
    As = np.asarray(lora_As, dtype=np.float32)
    Bs = np.asarray(lora_Bs, dtype=np.float32)
    idx = np.asarray(top_k_indices).astype(np.int64)
    vals = np.asarray(top_k_values, dtype=np.float32)

    i0, i1 = int(idx[0]), int(idx[1])
    v0, v1 = float(vals[0]), float(vals[1])

    wiT = np.ascontiguousarray(wi.T).astype(np.float16)          # [D, F]
    woT = np.ascontiguousarray(wo.T).astype(np.float16)          # [F, D]
    A0, A1 = As[i0], As[i1]                                      # [16, D]
    aT = np.ascontiguousarray(
        np.concatenate([A0, A1, A0], axis=0).T
    ).astype(np.float16)                                         # [D, 48]
    B0T, B1T = Bs[i0].T, Bs[i1].T                                # [16, F]
    bTa = np.zeros((128, D_FF), dtype=np.float16)
    bTa[0:16] = B0T.astype(np.float16)
    bTb = np.zeros((128, D_FF), dtype=np.float16)
    bTb[16:32] = B1T.astype(np.float16)
    bTb[32:48] = (-B0T).astype(np.float16)

    tokens = h.reshape(TOKENS, D_MODEL)
    n_cores = TOKENS // t_per_core
    in_maps = []
    for c in range(n_cores):
        shard = tokens[c * t_per_core:(c + 1) * t_per_core]
        xT = np.ascontiguousarray(shard.T).astype(np.float16)    # [D, Tc]
        in_maps.append({
            "xT": xT, "wiT": wiT, "woT": woT,
            "aT": aT, "bTa": bTa, "bTb": bTb,
        })
    return in_maps, v0, v1


# test.py can flip these to profile the run.
TRACE = False
TRACE_CORES = None
LAST_RESULT = None


def kernel(hidden_states, wi_w, wo_w, lora_As, lora_Bs,
           top_k_indices, top_k_values):
    global LAST_RESULT
    from concourse.bass_utils import run_bass_kernel_spmd

    in_maps, v0, v1 = prep_inputs(
        hidden_states, wi_w, wo_w, lora_As, lora_Bs,
        top_k_indices, top_k_values,
    )
    nc = _get_program(v0, v1)
    res = run_bass_kernel_spmd(
        nc, in_maps, list(range(N_CORES)),
        trace=TRACE, trace_cores=TRACE_CORES,
    )
    LAST_RESULT = res
    out = np.concatenate([r["out"] for r in res.results], axis=0)
    return out.reshape(B, S, D_MODEL).astype(np.float32, copy=False)
